# revision 9
# baseline (speedup 1.0000x reference)
"""Multi-head self-attention on 8 Trainium2 NeuronCores.

Problem: B=4, S=2048, D=1024, H=16 heads (head_dim 64), fp32.
  out = softmax((x Wq + bq)(x Wk + bk)^T / 8) (x Wv + bv) Wo + bo

Sharding: 8 shards = 4 batches x 2 head-groups (8 heads each).
Core c handles batch c//2, heads (c%2)*8 .. +8.  Wq/Wk/Wv column-sharded,
Wo row-sharded; each core emits a partial [S, D] output and the host sums
the two partials per batch + (bv @ Wo + bo).

Bias algebra (exact): bk shifts every logit in a query row equally ->
softmax-invariant -> dropped.  bv contributes (bv @ Wo) to every output
row (softmax weights sum to 1) -> folded into the host-side bias add.
Only bq stays on-chip (applied to Q at projection drain).

Per-core dataflow:
  Projections in float32r (full-rate fp32): x^T resident [D, S]; K^T and
  Q^T weight-stationary (k-outer over 2-t-tile groups so PE keeps pace
  with the x DMA), V x-stationary.  All drains convert to bf16: qts/kts
  [128, 4, S], vsb [128 sj, 16 sjt, 8 h, 65] with a ones column (PV then
  also produces softmax sums), valsT [128, 4, S].
  Attention per (si-block of 1024, head): logits^T tiles [sj 128, si
  1024] = 2 matmuls (lhsT = K^T chunk), exp on ScalarE (scale 1/8, no
  max subtraction: logits ~ N(0,1)) -> P^T bf16, PV accumulates
  [65, 1024] over 16 sj tiles.  Softmax sums (row 64) bounce
  PSUM->DRAM->SBUF[128,8] so the reciprocal runs on 128 partitions
  (DVE reciprocal is ~6.4 ns/elem on one partition), then
  DRAM-partition-broadcast to [64, 1024] and a fused
  normalize-multiply drains vals^T bf16 (odd heads DMA-shift into
  partitions 64:128).  Output projection per si-tile reuses the logits
  PSUM ring; partial [S, D] fp32 DMAs out.
"""
import numpy as np

B, S, D, H = 4, 2048, 1024, 16
HD = D // H          # 64
G = D // 2           # 512 columns per head-group
NCORES = 8
KT_ = 8              # D / 128 contraction tiles
TT = 4               # G / 128 dg tiles
ST = 16              # S / 128 s tiles
SB = 2               # si blocks
SBW = 1024           # si block width

_cache = {}


def _split_sync_waits(nc, mybir, max_waits=1):
    """walrus on this toolchain rejects >1 sem wait per instruction; move
    extra waits onto same-engine NoOps placed just before the instruction
    (engines are in-order, so this is semantics-preserving)."""
    for f in nc.m.functions:
        for bb in f.blocks:
            out, changed = [], False
            for inst in bb.instructions:
                si = inst.sync_info
                if si is not None and len(si.on_wait) > max_waits:
                    waits = list(si.on_wait)
                    head, tail = waits[:-max_waits], waits[-max_waits:]
                    for g in range(0, len(head), max_waits):
                        nop = mybir.InstNoOp(name=nc.get_next_instruction_name())
                        nop.engine = inst.engine
                        nop.sync_info = mybir.SyncInfo(
                            on_wait=head[g:g + max_waits], on_update=[])
                        nc.register_instruction(nop)
                        out.append(nop)
                    inst.sync_info = mybir.SyncInfo(
                        on_wait=tail, on_update=list(si.on_update))
                    changed = True
                out.append(inst)
            if changed:
                bb.instructions = out


def _build():
    import concourse.bass as bass
    import concourse.mybir as mybir
    import concourse.tile as tile

    F32 = mybir.dt.float32
    FR = mybir.dt.float32r
    BF16 = mybir.dt.bfloat16
    Exp = mybir.ActivationFunctionType.Exp

    nc = bass.Bass("TRN2", target_bir_lowering=False, debug=False,
                   num_devices=NCORES)
    xtd = nc.dram_tensor("xt", [D, S], FR, kind="ExternalInput")
    wqd = nc.dram_tensor("wq", [D, G], FR, kind="ExternalInput")
    wkd = nc.dram_tensor("wk", [D, G], FR, kind="ExternalInput")
    wvd = nc.dram_tensor("wv", [D, G], FR, kind="ExternalInput")
    wod = nc.dram_tensor("wo", [G, D], BF16, kind="ExternalInput")
    bqd = nc.dram_tensor("bq", [G], F32, kind="ExternalInput")
    outd = nc.dram_tensor("out", [S, D], F32, kind="ExternalOutput")

    with tile.TileContext(nc) as tc:
        with tc.tile_pool(name="persist", bufs=1) as pp, \
             tc.tile_pool(name="dram", bufs=1, space="DRAM") as dp:
            qts = pp.tile([128, TT, S], BF16, tag="qts")
            kts = pp.tile([128, TT, S], BF16, tag="kts")
            vsb = pp.tile([128, ST, 8, HD + 1], BF16, tag="vsb")
            valsT = pp.tile([128, TT, S], BF16, tag="valsT")
            wos = pp.tile([128, TT, D], BF16, tag="wos")
            bqt = pp.tile([128, TT], F32, tag="bqt")

            nc.vector.memset(vsb[:, :, :, HD:HD + 1], 1.0)

            # ---- Phase 1: projections (fp32r, drains to bf16) ----
            with tc.tile_pool(name="proj", bufs=1) as jp, \
                 tc.tile_pool(name="ps_proj", bufs=8, space="PSUM") as psp:
                xts = jp.tile([128, KT_, S], FR, tag="xts")
                wqs = jp.tile([128, KT_, G], FR, tag="wqs")
                wks = jp.tile([128, KT_, G], FR, tag="wks")
                wvs = jp.tile([128, KT_, G], FR, tag="wvs")
                # stagger loads so K-proj never outruns the x stream
                for k in range(KT_):
                    nc.sync.dma_start(out=wks[:, k, :], in_=wkd[k * 128:(k + 1) * 128, :])
                    nc.sync.dma_start(out=xts[:, k, :], in_=xtd[k * 128:(k + 1) * 128, :])
                for k in range(KT_):
                    nc.sync.dma_start(out=wvs[:, k, :], in_=wvd[k * 128:(k + 1) * 128, :])
                for k in range(KT_):
                    nc.sync.dma_start(out=wqs[:, k, :], in_=wqd[k * 128:(k + 1) * 128, :])
                for t in range(TT):
                    nc.sync.dma_start(out=wos[:, t, :], in_=wod[t * 128:(t + 1) * 128, :])
                # bq is host-pretransposed to [128, 4] p-major (16B/partition
                # descriptors); keep this small gather off the sync queue and
                # behind the big loads so it can't head-of-line block them.
                nc.gpsimd.dma_start(
                    out=bqt, in_=bqd.rearrange("(p t) -> p t", t=TT))

                # K^T then Q^T: weight-stationary, 2-t-tile groups, k-outer
                for which, ws, dst in (("k", wks, kts), ("q", wqs, qts)):
                    for tg in range(2):
                        pss = [[psp.tile([128, 512], F32, tag="pj", name="pj")
                                for _ in range(4)] for _ in range(2)]
                        for k in range(KT_):
                            for ti in range(2):
                                t = 2 * tg + ti
                                for sc in range(4):
                                    nc.tensor.matmul(
                                        pss[ti][sc],
                                        ws[:, k, t * 128:(t + 1) * 128],
                                        xts[:, k, sc * 512:(sc + 1) * 512],
                                        start=(k == 0), stop=(k == KT_ - 1))
                        for ti in range(2):
                            t = 2 * tg + ti
                            for sc in range(4):
                                d_ = dst[:, t, sc * 512:(sc + 1) * 512]
                                if which == "q":
                                    nc.vector.tensor_scalar_add(
                                        d_, pss[ti][sc], bqt[:, t:t + 1])
                                else:
                                    nc.vector.tensor_copy(out=d_, in_=pss[ti][sc])

                # V: x-stationary
                for s_ in range(ST):
                    ps = psp.tile([128, 512], F32, tag="pj", name="pj")
                    for k in range(KT_):
                        nc.tensor.matmul(
                            ps, xts[:, k, s_ * 128:(s_ + 1) * 128],
                            wvs[:, k, :],
                            start=(k == 0), stop=(k == KT_ - 1))
                    nc.vector.tensor_copy(
                        out=vsb[:, s_, :, 0:HD],
                        in_=ps.rearrange("p (h d) -> p h d", h=8))

            # ---- Phase 2: attention + output projection ----
            # Software-pipelined emission: PV(unit) is emitted AFTER
            # logits(unit+1) so the in-order PE never sits behind the exp
            # it needs (that serialization held the baseline PE at ~1 GHz
            # p-state).  4 of 16 exp tiles per head run as a Schraudolph
            # bf16 fast-exp on DVE so ScalarE stays just below PE speed.
            # Per-head normalize chains and the previous block's output
            # projection are interleaved into the following head's units.
            DVE_SJ = (2, 7, 10, 14)
            # Schraudolph: e^(l/8) = 2^(l*0.125*log2 e); bf16 bits =
            # 128*(z+127) - 5.5 (centers the pwl error) + 0.5 (truncation)
            FE_A = 0.125 * 1.4426950408889634 * 128.0
            FE_B = 127.0 * 128.0 - 5.5 + 0.5
            with tc.tile_pool(name="ptp", bufs=4) as ptp, \
                 tc.tile_pool(name="bcp", bufs=2) as bcp, \
                 tc.tile_pool(name="smp", bufs=2) as smp, \
                 tc.tile_pool(name="vshift", bufs=2) as vsp, \
                 tc.tile_pool(name="outp", bufs=2) as op_, \
                 tc.tile_pool(name="ps_lg", bufs=2, space="PSUM") as lgp, \
                 tc.tile_pool(name="ps_pv", bufs=2, space="PSUM") as pvp:

                def emit_chain_part(part, h, blk, pv):
                    """normalize chain for (h, blk), split into 3 stages"""
                    t, p0 = h // 2, (h % 2) * 64
                    s0b = blk * SBW
                    if part == 0:
                        srow_sb = smp.tile([1, SBW], F32, tag="srow_sb")
                        nc.vector.tensor_copy(out=srow_sb, in_=pv[64:65, :])
                        srow = dp.tile([SBW], F32, tag="srow", bufs=3)
                        nc.gpsimd.dma_start(
                            out=srow.rearrange("(a b) -> a b", a=1), in_=srow_sb)
                        ssb = smp.tile([128, SBW // 128], F32, tag="ssb")
                        nc.gpsimd.dma_start(
                            out=ssb, in_=srow.rearrange("(p f) -> p f", p=128))
                        return ssb
                    if part == 1:
                        ssb = pv  # stage handle
                        rsb = smp.tile([128, SBW // 128], F32, tag="rsb")
                        nc.vector.reciprocal(out=rsb, in_=ssb)
                        rrow = dp.tile([SBW], F32, tag="rrow", bufs=3)
                        nc.gpsimd.dma_start(
                            out=rrow.rearrange("(p f) -> p f", p=128), in_=rsb)
                        bc = bcp.tile([64, SBW], F32, tag="bc")
                        nc.gpsimd.dma_start(
                            out=bc,
                            in_=rrow.rearrange("(a b) -> a b", a=1)
                                    .partition_broadcast(64))
                        return bc
                    # part 2: normalize multiply (+ shift for odd heads)
                    pv_t, bc = pv
                    if p0 == 0:
                        nc.vector.tensor_mul(
                            valsT[0:64, t, s0b:s0b + SBW], pv_t[0:64, :], bc)
                    else:
                        vs = vsp.tile([64, SBW], BF16, tag="vs")
                        nc.vector.tensor_mul(vs, pv_t[0:64, :], bc)
                        nc.gpsimd.dma_start(
                            out=valsT[64:128, t, s0b:s0b + SBW], in_=vs)
                    return None

                def emit_outproj_unit(st, blk):
                    s0 = blk * SBW + st * 128
                    ops = lgp.tile([128, SBW], F32, tag="lg", name="ops")
                    for gt in range(TT):
                        for hf in range(2):
                            nc.tensor.matmul(
                                ops[:, hf * 512:(hf + 1) * 512],
                                valsT[:, gt, s0:s0 + 128],
                                wos[:, gt, hf * 512:(hf + 1) * 512],
                                start=(gt == 0), stop=(gt == TT - 1))
                    ob = op_.tile([128, D], F32, tag="ob")
                    nc.vector.tensor_copy(out=ob, in_=ops)
                    nc.gpsimd.dma_start(out=outd[s0:s0 + 128, :], in_=ob)

                pending_pv = None      # () -> None, PV matmuls one unit behind
                chain = None           # (h, blk, pv) awaiting normalize chain
                chain_stage = None
                outproj_q = []         # deferred outproj units (st, blk)
                for hs in range(2 * 8):
                    blk, h = hs // 8, hs % 8
                    s0b = blk * SBW
                    t, p0 = h // 2, (h % 2) * 64
                    qrow = qts[p0:p0 + 64, t, s0b:s0b + SBW]
                    pv = pvp.tile([65, SBW], F32, tag="pv")
                    for sj in range(ST):
                        lg = lgp.tile([128, SBW], F32, tag="lg")
                        lkt = kts[p0:p0 + 64, t, sj * 128:(sj + 1) * 128]
                        for hf in range(2):
                            nc.tensor.matmul(
                                lg[:, hf * 512:(hf + 1) * 512],
                                lkt, qrow[:, hf * 512:(hf + 1) * 512],
                                start=True, stop=True)
                        pt = ptp.tile([128, SBW], BF16, tag="pt")
                        if sj in DVE_SJ:
                            nc.vector.tensor_scalar(
                                pt.bitcast(mybir.dt.int16), lg, FE_A, FE_B,
                                mybir.AluOpType.mult, mybir.AluOpType.add)
                        else:
                            nc.scalar.activation(pt, lg, Exp, scale=0.125)
                        if pending_pv is not None:
                            pending_pv()
                        def _pv(pv=pv, pt=pt, sj=sj, h=h):
                            lv = vsb[:, sj, h, 0:HD + 1]
                            for hf in range(2):
                                nc.tensor.matmul(
                                    pv[:, hf * 512:(hf + 1) * 512],
                                    lv, pt[:, hf * 512:(hf + 1) * 512],
                                    start=(sj == 0), stop=(sj == ST - 1))
                        pending_pv = _pv
                        # interleave previous head's normalize chain
                        if chain is not None:
                            ch_h, ch_blk, ch_pv = chain
                            if sj == 1:
                                chain_stage = emit_chain_part(0, ch_h, ch_blk, ch_pv)
                            elif sj == 3:
                                chain_stage = emit_chain_part(1, ch_h, ch_blk, chain_stage)
                            elif sj == 6:
                                emit_chain_part(2, ch_h, ch_blk, (ch_pv, chain_stage))
                                chain = None
                                if ch_h == 7 and ch_blk == 0:
                                    outproj_q = [(st, 0) for st in range(SBW // 128)]
                        # drip the deferred output projection into this stream
                        if outproj_q and sj % 2 == 0 and sj >= 10:
                            emit_outproj_unit(*outproj_q.pop(0))
                    pending_pv()
                    pending_pv = None
                    chain = (h, blk, pv)
                # tail: last head's chain, then the last block's outproj
                chain_stage = emit_chain_part(0, 7, 1, chain[2])
                chain_stage = emit_chain_part(1, 7, 1, chain_stage)
                emit_chain_part(2, 7, 1, (chain[2], chain_stage))
                for st in range(SBW // 128):
                    emit_outproj_unit(st, 1)

    _split_sync_waits(nc, mybir)
    return nc


def _get_nc():
    if "nc" not in _cache:
        _cache["nc"] = _build()
    return _cache["nc"]


def _run(in_maps, **kw):
    from concourse.bass_utils import run_bass_kernel_spmd
    return run_bass_kernel_spmd(_get_nc(), in_maps, core_ids=list(range(NCORES)), **kw)


def _make_in_maps(x, Wq, bq, Wk, bk, Wv, bv, Wo, bo):
    import ml_dtypes
    x = np.asarray(x, np.float32)
    Wo32 = np.asarray(Wo, np.float32)
    in_maps = []
    for c in range(NCORES):
        b, g = c // 2, c % 2
        gs = slice(g * G, (g + 1) * G)
        in_maps.append({
            "xt": np.ascontiguousarray(x[b].T),
            "wq": np.ascontiguousarray(np.asarray(Wq, np.float32)[:, gs]),
            "wk": np.ascontiguousarray(np.asarray(Wk, np.float32)[:, gs]),
            "wv": np.ascontiguousarray(np.asarray(Wv, np.float32)[:, gs]),
            "wo": np.ascontiguousarray(Wo32[gs, :].astype(ml_dtypes.bfloat16)),
            # pre-transposed to [128 partitions, 4 t-tiles] p-major so the
            # on-chip gather is 16B-per-partition descriptors, not 4B
            "bq": np.ascontiguousarray(
                np.asarray(bq, np.float32)[gs].reshape(TT, 128).T),
        })
    return in_maps


def kernel(x, Wq, bq, Wk, bk, Wv, bv, Wo, bo, **_kw):
    res = _run(_make_in_maps(x, Wq, bq, Wk, bk, Wv, bv, Wo, bo))
    # host-side bias: bv @ Wo + bo (exact: softmax rows sum to 1)
    hb = (np.asarray(bv, np.float64) @ np.asarray(Wo, np.float64)
          + np.asarray(bo, np.float64)).astype(np.float32)
    out = np.empty((B, S, D), dtype=np.float32)
    for b in range(B):
        out[b] = res.results[2 * b]["out"] + res.results[2 * b + 1]["out"] + hb
    return out


# revision 23
# speedup vs baseline: 1.0795x; 1.0795x over previous
"""Multi-head self-attention on 8 Trainium2 NeuronCores.

Problem: B=4, S=2048, D=1024, H=16 heads (head_dim 64), fp32.
  out = softmax((x Wq + bq)(x Wk + bk)^T / 8) (x Wv + bv) Wo + bo

Sharding: 8 shards = 4 batches x 2 head-groups (8 heads each).
Core c handles batch c//2, heads (c%2)*8 .. +8.  Wq/Wk/Wv column-sharded,
Wo row-sharded; each core emits a partial [S, D] output and the host sums
the two partials per batch + (bv @ Wo + bo).

Bias algebra (exact): bk shifts every logit in a query row equally ->
softmax-invariant -> dropped.  bv contributes (bv @ Wo) to every output
row (softmax weights sum to 1) -> folded into the host-side bias add.
Only bq stays on-chip (applied at the Q projection drain).

Per-core dataflow (all matmuls bf16 -> fp32 PSUM; fp8 projections were
tried and fail the 2e-2 budget at ~5.5e-2):
  Projections from bf16 x^T resident [D, S]: K^T and Q^T weight-
  stationary (k-outer over 2-t-tile groups so the PE paces the x DMA),
  V x-stationary.  Drains: Q -> ScalarE (fused bias), K -> DVE, V ->
  ScalarE; all to bf16: qts/kts [128, 4, S], vsb [128 sj, 16 sjt, 8 h,
  65] with a ones column so PV also produces the softmax sums.
  Attention per (si-block 1024, head): logits^T tiles [sj 128, si 1024]
  (lhsT = K^T chunk).  Each exp tile is split in half so its latency
  stays under the PE's 2-unit pipeline slack: ScalarE table-exp on
  columns 0:512 (scale 1/8; no max subtraction, logits ~ N(0,1)) and a
  DVE Schraudolph bf16 bit-trick exp (int16 mult-add, bitcast bf16,
  pwl error ~+-3%, cancels to first order in the normalize) on
  512:1024.  Each PV half-matmul depends only on its own half.  PV
  accumulates [65, 1024] over 16 sj tiles.  Emission is software-
  pipelined: PV(unit) is emitted after logits(unit+2), so the in-order
  PE never sits behind the exp it needs — the p-state clock collapses
  to half speed otherwise.  Softmax sums (PSUM row 64) are copied out
  in two 512-halves on ScalarE, bounced DRAM -> SBUF [128, 8]
  (reciprocal on one partition costs ~6.4 ns/elem, on 128 it is
  free), reciprocal on DVE, broadcast back to [64, 1024] via a DRAM
  partition-replicating DMA (bounce DMAs ride the otherwise-idle Sync
  queue), then two fused normalize-multiply halves on DVE drain
  vals^T bf16 (odd heads DMA-shift into partitions 64:128).  The
  block-0 output projection is dripped one si-tile per head into
  block 1's unit stream (its PSUM reuses the logits ring, its drain
  runs as two ScalarE halves); block 1's runs at the tail.  Partial
  [S, D] fp32 DMAs out per si-tile.
"""
import numpy as np

B, S, D, H = 4, 2048, 1024, 16
HD = D // H          # 64
G = D // 2           # 512 columns per head-group
NCORES = 8
KT_ = 8              # D / 128 contraction tiles
TT = 4               # G / 128 dg tiles
ST = 16              # S / 128 s tiles
SB = 2               # si blocks
SBW = 1024           # si block width

_cache = {}


def _split_sync_waits(nc, mybir, max_waits=1):
    """walrus on this toolchain rejects >1 sem wait per instruction; move
    extra waits onto same-engine NoOps placed just before the instruction
    (engines are in-order, so this is semantics-preserving)."""
    for f in nc.m.functions:
        for bb in f.blocks:
            out, changed = [], False
            for inst in bb.instructions:
                si = inst.sync_info
                if si is not None and len(si.on_wait) > max_waits:
                    waits = list(si.on_wait)
                    head, tail = waits[:-max_waits], waits[-max_waits:]
                    for g in range(0, len(head), max_waits):
                        nop = mybir.InstNoOp(name=nc.get_next_instruction_name())
                        nop.engine = inst.engine
                        nop.sync_info = mybir.SyncInfo(
                            on_wait=head[g:g + max_waits], on_update=[])
                        nc.register_instruction(nop)
                        out.append(nop)
                    inst.sync_info = mybir.SyncInfo(
                        on_wait=tail, on_update=list(si.on_update))
                    changed = True
                out.append(inst)
            if changed:
                bb.instructions = out


def _build():
    import concourse.bass as bass
    import concourse.mybir as mybir
    import concourse.tile as tile

    F32 = mybir.dt.float32
    BF16 = mybir.dt.bfloat16
    I16 = mybir.dt.int16
    Exp = mybir.ActivationFunctionType.Exp
    Ident = mybir.ActivationFunctionType.Identity
    MUL = mybir.AluOpType.mult
    ADD = mybir.AluOpType.add

    # Schraudolph: e^(l/8) = 2^(l*0.125*log2 e); bf16 bits =
    # 128*(z+127) - 5.5 (centers the pwl error) + 0.5 (truncation)
    FE_A = 0.125 * 1.4426950408889634 * 128.0
    FE_B = 127.0 * 128.0 - 5.5 + 0.5

    nc = bass.Bass("TRN2", target_bir_lowering=False, debug=False,
                   num_devices=NCORES)
    xbd = nc.dram_tensor("xb", [D, S], BF16, kind="ExternalInput")
    wqbd = nc.dram_tensor("wqb", [D, G], BF16, kind="ExternalInput")
    wkbd = nc.dram_tensor("wkb", [D, G], BF16, kind="ExternalInput")
    wvbd = nc.dram_tensor("wvb", [D, G], BF16, kind="ExternalInput")
    wod = nc.dram_tensor("wo", [G, D], BF16, kind="ExternalInput")
    bqd = nc.dram_tensor("bq", [G], F32, kind="ExternalInput")
    outd = nc.dram_tensor("out", [S, D], F32, kind="ExternalOutput")

    with tile.TileContext(nc) as tc:
        with tc.tile_pool(name="persist", bufs=1) as pp, \
             tc.tile_pool(name="dram", bufs=1, space="DRAM") as dp:
            qts = pp.tile([128, TT, S], BF16, tag="qts")
            kts = pp.tile([128, TT, S], BF16, tag="kts")
            vsb = pp.tile([128, ST, 8, HD + 1], BF16, tag="vsb")
            valsT = pp.tile([128, TT, S], BF16, tag="valsT")
            wos = pp.tile([128, TT, D], BF16, tag="wos")
            bqt = pp.tile([128, TT], F32, tag="bqt")

            nc.vector.memset(vsb[:, :, :, HD:HD + 1], 1.0)

            # ---- Phase 1: projections (bf16) ----
            with tc.tile_pool(name="proj", bufs=1) as jp, \
                 tc.tile_pool(name="ps_proj", bufs=8, space="PSUM") as psp:
                xbs = jp.tile([128, KT_, S], BF16, tag="xbs")
                wqbs = jp.tile([128, KT_, G], BF16, tag="wqbs")
                wkbs = jp.tile([128, KT_, G], BF16, tag="wkbs")
                wvbs = jp.tile([128, KT_, G], BF16, tag="wvbs")
                # stagger loads so K-proj never outruns the x stream
                for k in range(KT_):
                    nc.sync.dma_start(out=wkbs[:, k, :], in_=wkbd[k * 128:(k + 1) * 128, :])
                    nc.sync.dma_start(out=xbs[:, k, :], in_=xbd[k * 128:(k + 1) * 128, :])
                for k in range(KT_):
                    nc.sync.dma_start(out=wqbs[:, k, :], in_=wqbd[k * 128:(k + 1) * 128, :])
                for k in range(KT_):
                    nc.sync.dma_start(out=wvbs[:, k, :], in_=wvbd[k * 128:(k + 1) * 128, :])
                for t in range(TT):
                    nc.sync.dma_start(out=wos[:, t, :], in_=wod[t * 128:(t + 1) * 128, :])
                # bq host-pretransposed to [128, 4] p-major; small gather
                # stays off the sync queue and behind the big loads
                nc.gpsimd.dma_start(
                    out=bqt, in_=bqd.rearrange("(p t) -> p t", t=TT))

                # K^T then Q^T: weight-stationary, 2-t-tile groups, k-outer
                for which, ws in (("k", wkbs), ("q", wqbs)):
                    for tg in range(2):
                        pss = [[psp.tile([128, 512], F32, tag="pj", name="pj")
                                for _ in range(4)] for _ in range(2)]
                        for k in range(KT_):
                            for ti in range(2):
                                t = 2 * tg + ti
                                for sc in range(4):
                                    nc.tensor.matmul(
                                        pss[ti][sc],
                                        ws[:, k, t * 128:(t + 1) * 128],
                                        xbs[:, k, sc * 512:(sc + 1) * 512],
                                        start=(k == 0), stop=(k == KT_ - 1))
                        for ti in range(2):
                            t = 2 * tg + ti
                            for sc in range(4):
                                ps_, sl = pss[ti][sc], slice(sc * 512, (sc + 1) * 512)
                                if which == "q":
                                    nc.scalar.activation(
                                        qts[:, t, sl], ps_, Ident,
                                        bias=bqt[:, t:t + 1], scale=1.0)
                                else:
                                    nc.vector.tensor_copy(out=kts[:, t, sl], in_=ps_)

                # V: x-stationary; drains on ScalarE (idle in proj)
                for s_ in range(ST):
                    ps = psp.tile([128, 512], F32, tag="pj", name="pj")
                    for k in range(KT_):
                        nc.tensor.matmul(
                            ps, xbs[:, k, s_ * 128:(s_ + 1) * 128],
                            wvbs[:, k, :],
                            start=(k == 0), stop=(k == KT_ - 1))
                    nc.scalar.copy(
                        out=vsb[:, s_, :, 0:HD],
                        in_=ps.rearrange("p (h d) -> p h d", h=8))

            # ---- Phase 2: attention + output projection ----
            with tc.tile_pool(name="ptp", bufs=4) as ptp, \
                 tc.tile_pool(name="bcp", bufs=2) as bcp, \
                 tc.tile_pool(name="smp", bufs=2) as smp, \
                 tc.tile_pool(name="vshift", bufs=2) as vsp, \
                 tc.tile_pool(name="outp", bufs=2) as op_, \
                 tc.tile_pool(name="ps_lg", bufs=2, space="PSUM") as lgp, \
                 tc.tile_pool(name="ps_pv", bufs=2, space="PSUM") as pvp:

                def chain_srow_half(state, hf):
                    """copy half the sums row (PSUM -> SBUF, ScalarE)"""
                    pv_t = state["pv"]
                    if "srow_sb" not in state:
                        state["srow_sb"] = smp.tile([1, SBW], F32, tag="srow_sb", name="srow_sb")
                        state["srow"] = dp.tile([SBW], F32, tag="srow", bufs=3, name="srow")
                    sl = slice(hf * 512, (hf + 1) * 512)
                    nc.scalar.copy(out=state["srow_sb"][:, sl], in_=pv_t[64:65, sl])
                    nc.sync.dma_start(
                        out=state["srow"].rearrange("(a b) -> a b", a=1)[:, sl],
                        in_=state["srow_sb"][:, sl])

                def chain_recip(state):
                    ssb = smp.tile([128, SBW // 128], F32, tag="ssb")
                    nc.sync.dma_start(
                        out=ssb, in_=state["srow"].rearrange("(p f) -> p f", p=128))
                    rsb = smp.tile([128, SBW // 128], F32, tag="rsb")
                    nc.vector.reciprocal(out=rsb, in_=ssb)
                    rrow = dp.tile([SBW], F32, tag="rrow", bufs=3)
                    nc.sync.dma_start(
                        out=rrow.rearrange("(p f) -> p f", p=128), in_=rsb)
                    bc = bcp.tile([64, SBW], F32, tag="bc")
                    nc.sync.dma_start(
                        out=bc,
                        in_=rrow.rearrange("(a b) -> a b", a=1)
                                .partition_broadcast(64))
                    state["bc"] = bc

                def chain_norm_half(state, hf):
                    """normalize-multiply half on DVE (+ stage odd heads)"""
                    h, blk, pv_t, bc = state["h"], state["blk"], state["pv"], state["bc"]
                    t, p0 = h // 2, (h % 2) * 64
                    s0b = blk * SBW
                    sl = slice(hf * 512, (hf + 1) * 512)
                    osl = slice(s0b + hf * 512, s0b + (hf + 1) * 512)
                    if p0 == 0:
                        nc.vector.tensor_mul(
                            valsT[0:64, t, osl], pv_t[0:64, sl], bc[:, sl])
                    else:
                        if "vs" not in state:
                            state["vs"] = vsp.tile([64, SBW], BF16, tag="vs", name="vs")
                        nc.vector.tensor_mul(
                            state["vs"][:, sl], pv_t[0:64, sl], bc[:, sl])
                        if hf == 1:
                            nc.gpsimd.dma_start(
                                out=valsT[64:128, t, s0b:s0b + SBW],
                                in_=state["vs"])

                def outproj_mm(state, half):
                    """output projection si-tile: 4 accumulating matmuls"""
                    if state.get("ops") is None:
                        state["ops"] = lgp.tile([128, SBW], F32, tag="lg", name="ops")
                    s0 = state["blk"] * SBW + state["st"] * 128
                    for gt in range(TT):
                        nc.tensor.matmul(
                            state["ops"][:, half * 512:(half + 1) * 512],
                            valsT[:, gt, s0:s0 + 128],
                            wos[:, gt, half * 512:(half + 1) * 512],
                            start=(gt == 0), stop=(gt == TT - 1))

                def outproj_drain_half(state, hf):
                    if state.get("ob") is None:
                        state["ob"] = op_.tile([128, D], F32, tag="ob", name="ob")
                    sl = slice(hf * 512, (hf + 1) * 512)
                    nc.scalar.copy(out=state["ob"][:, sl], in_=state["ops"][:, sl])
                    if hf == 1:
                        s0 = state["blk"] * SBW + state["st"] * 128
                        nc.sync.dma_start(out=outd[s0:s0 + 128, :], in_=state["ob"])

                pending_pv = []        # PV emitters, two units behind logits
                chain = None           # state dict for previous head's chain
                outproj_q = []         # deferred outproj units (st, blk)
                op_state = None
                for hs in range(2 * 8):
                    blk, h = hs // 8, hs % 8
                    s0b = blk * SBW
                    t, p0 = h // 2, (h % 2) * 64
                    qrow = qts[p0:p0 + 64, t, s0b:s0b + SBW]
                    pv = pvp.tile([65, SBW], F32, tag="pv")
                    for sj in range(ST):
                        lg = lgp.tile([128, SBW], F32, tag="lg")
                        lkt = kts[p0:p0 + 64, t, sj * 128:(sj + 1) * 128]
                        for hf in range(2):
                            nc.tensor.matmul(
                                lg[:, hf * 512:(hf + 1) * 512],
                                lkt, qrow[:, hf * 512:(hf + 1) * 512],
                                start=True, stop=True)
                        pt = ptp.tile([128, SBW], BF16, tag="pt")
                        # exp split: ScalarE table-exp half + DVE fast-exp
                        # half; each PV half waits only on its own half
                        nc.scalar.activation(
                            pt[:, 0:512], lg[:, 0:512], Exp, scale=0.125)
                        nc.vector.tensor_scalar(
                            pt.bitcast(I16)[:, 512:1024], lg[:, 512:1024],
                            FE_A, FE_B, MUL, ADD)
                        def _pv(pv=pv, pt=pt, sj=sj, h=h):
                            lv = vsb[:, sj, h, 0:HD + 1]
                            for hf in range(2):
                                nc.tensor.matmul(
                                    pv[:, hf * 512:(hf + 1) * 512],
                                    lv, pt[:, hf * 512:(hf + 1) * 512],
                                    start=(sj == 0), stop=(sj == ST - 1))
                        pending_pv.append(_pv)
                        if len(pending_pv) > 2:
                            pending_pv.pop(0)()
                        # previous head's normalize chain, spread thin
                        if chain is not None:
                            if sj == 8:
                                chain_srow_half(chain, 0)
                            elif sj == 10:
                                chain_srow_half(chain, 1)
                            elif sj == 12:
                                chain_recip(chain)
                            elif sj == 14:
                                chain_norm_half(chain, 0)
                            elif sj == 15:
                                chain_norm_half(chain, 1)
                                if chain["h"] == 7 and chain["blk"] == 0:
                                    outproj_q = [(st, 0) for st in range(SBW // 128)]
                                chain = None
                        # drip the deferred output projection, split thin
                        if sj == 2 and outproj_q and op_state is None:
                            st_, blk_ = outproj_q.pop(0)
                            op_state = {"st": st_, "blk": blk_}
                            outproj_mm(op_state, 0)
                        elif sj == 4 and op_state is not None:
                            outproj_mm(op_state, 1)
                        elif sj == 5 and op_state is not None:
                            outproj_drain_half(op_state, 0)
                        elif sj == 7 and op_state is not None:
                            outproj_drain_half(op_state, 1)
                            op_state = None
                    chain = {"h": h, "blk": blk, "pv": pv}
                for f in pending_pv:   # flush the last two PV units
                    f()
                pending_pv = []
                # tail: last head's chain, then the last block's outproj
                chain_srow_half(chain, 0)
                chain_srow_half(chain, 1)
                chain_recip(chain)
                chain_norm_half(chain, 0)
                chain_norm_half(chain, 1)
                tail_units = ([{"st": st_, "blk": blk_} for st_, blk_ in outproj_q]
                              + [{"st": st, "blk": 1} for st in range(SBW // 128)])
                for st_state in tail_units:
                    outproj_mm(st_state, 0)
                    outproj_mm(st_state, 1)
                    outproj_drain_half(st_state, 0)
                    outproj_drain_half(st_state, 1)

    _split_sync_waits(nc, mybir)
    return nc


def _get_nc():
    if "nc" not in _cache:
        _cache["nc"] = _build()
    return _cache["nc"]


def _run(in_maps, **kw):
    from concourse.bass_utils import run_bass_kernel_spmd
    return run_bass_kernel_spmd(_get_nc(), in_maps, core_ids=list(range(NCORES)), **kw)


def _make_in_maps(x, Wq, bq, Wk, bk, Wv, bv, Wo, bo):
    import ml_dtypes
    BF = ml_dtypes.bfloat16
    x = np.asarray(x, np.float32)
    in_maps = []
    for c in range(NCORES):
        b, g = c // 2, c % 2
        gs = slice(g * G, (g + 1) * G)
        in_maps.append({
            "xb": np.ascontiguousarray(x[b].T).astype(BF),
            "wqb": np.ascontiguousarray(np.asarray(Wq, np.float32)[:, gs]).astype(BF),
            "wkb": np.ascontiguousarray(np.asarray(Wk, np.float32)[:, gs]).astype(BF),
            "wvb": np.ascontiguousarray(np.asarray(Wv, np.float32)[:, gs]).astype(BF),
            "wo": np.ascontiguousarray(np.asarray(Wo, np.float32)[gs, :]).astype(BF),
            # pre-transposed to [128 partitions, 4 t-tiles] p-major
            "bq": np.ascontiguousarray(
                np.asarray(bq, np.float32)[gs].reshape(TT, 128).T),
        })
    return in_maps


def kernel(x, Wq, bq, Wk, bk, Wv, bv, Wo, bo, **_kw):
    res = _run(_make_in_maps(x, Wq, bq, Wk, bk, Wv, bv, Wo, bo))
    # host-side bias: bv @ Wo + bo (exact: softmax rows sum to 1)
    hb = (np.asarray(bv, np.float64) @ np.asarray(Wo, np.float64)
          + np.asarray(bo, np.float64)).astype(np.float32)
    out = np.empty((B, S, D), dtype=np.float32)
    for b in range(B):
        out[b] = res.results[2 * b]["out"] + res.results[2 * b + 1]["out"] + hb
    return out


# revision 24
# speedup vs baseline: 1.7400x; 1.6118x over previous
"""Multi-head self-attention on 8 Trainium2 NeuronCores.

Problem: B=4, S=2048, D=1024, H=16 heads (head_dim 64), fp32.
  out = softmax((x Wq + bq)(x Wk + bk)^T / 8) (x Wv + bv) Wo + bo

Sharding: 8 shards = 4 batches x 2 head-groups (8 heads each).
Core c handles batch c//2, heads (c%2)*8 .. +8.  Wq/Wk/Wv column-sharded,
Wo row-sharded; each core emits a partial [S, D] output and the host sums
the two partials per batch + (bv @ Wo + bo).

Bias algebra (exact): bk shifts every logit in a query row equally ->
softmax-invariant -> dropped.  bv contributes (bv @ Wo) to every output
row (softmax weights sum to 1) -> folded into the host-side bias add.
Only bq stays on-chip (applied at the Q projection drain).

Per-core dataflow (all matmuls bf16 -> fp32 PSUM; fp8 projections were
tried and fail the 2e-2 budget at ~5.5e-2):
  Projections from bf16 x^T resident [D, S]: K^T and Q^T weight-
  stationary (k-outer over 2-t-tile groups so the PE paces the x DMA),
  V x-stationary.  Drains: Q -> ScalarE (fused bias), K -> DVE, V ->
  ScalarE; all to bf16: qts/kts [128, 4, S], vsb [128 sj, 16 sjt, 8 h,
  65] with a ones column so PV also produces the softmax sums.
  Attention per (si-block 1024, head): logits^T tiles [sj 128, si 1024]
  (lhsT = K^T chunk).  Each exp tile is split in half so its latency
  stays under the PE's 2-unit pipeline slack: ScalarE table-exp on
  columns 0:512 (scale 1/8; no max subtraction, logits ~ N(0,1)) and a
  DVE Schraudolph bf16 bit-trick exp (int16 mult-add, bitcast bf16,
  pwl error ~+-3%, cancels to first order in the normalize) on
  512:1024.  Each PV half-matmul depends only on its own half.  PV
  accumulates [65, 1024] over 16 sj tiles.  Emission is software-
  pipelined: PV(unit) is emitted after logits(unit+2), so the in-order
  PE never sits behind the exp it needs — the p-state clock collapses
  to half speed otherwise.  Softmax sums (PSUM row 64) are copied out
  in two 512-halves on ScalarE, bounced DRAM -> SBUF [128, 8]
  (reciprocal on one partition costs ~6.4 ns/elem, on 128 it is
  free), reciprocal on DVE, broadcast back to [64, 1024] via a DRAM
  partition-replicating DMA (bounce DMAs ride the otherwise-idle Sync
  queue), then two fused normalize-multiply halves on DVE drain
  vals^T bf16 (odd heads DMA-shift into partitions 64:128).  The
  block-0 output projection is dripped one si-tile per head into
  block 1's unit stream (its PSUM reuses the logits ring, its drain
  runs as two ScalarE halves); block 1's runs at the tail.  Partial
  [S, D] fp32 DMAs out per si-tile.
"""
import numpy as np

B, S, D, H = 4, 2048, 1024, 16
HD = D // H          # 64
G = D // 2           # 512 columns per head-group
NCORES = 8
KT_ = 8              # D / 128 contraction tiles
TT = 4               # G / 128 dg tiles
ST = 16              # S / 128 s tiles
SB = 2               # si blocks
SBW = 1024           # si block width

_cache = {}


def _split_sync_waits(nc, mybir, max_waits=1):
    """walrus on this toolchain rejects >1 sem wait per instruction; move
    extra waits onto same-engine NoOps placed just before the instruction
    (engines are in-order, so this is semantics-preserving)."""
    for f in nc.m.functions:
        for bb in f.blocks:
            out, changed = [], False
            for inst in bb.instructions:
                si = inst.sync_info
                if si is not None and len(si.on_wait) > max_waits:
                    waits = list(si.on_wait)
                    head, tail = waits[:-max_waits], waits[-max_waits:]
                    for g in range(0, len(head), max_waits):
                        nop = mybir.InstNoOp(name=nc.get_next_instruction_name())
                        nop.engine = inst.engine
                        nop.sync_info = mybir.SyncInfo(
                            on_wait=head[g:g + max_waits], on_update=[])
                        nc.register_instruction(nop)
                        out.append(nop)
                    inst.sync_info = mybir.SyncInfo(
                        on_wait=tail, on_update=list(si.on_update))
                    changed = True
                out.append(inst)
            if changed:
                bb.instructions = out


def _build():
    import concourse.bass as bass
    import concourse.mybir as mybir
    import concourse.tile as tile

    F32 = mybir.dt.float32
    BF16 = mybir.dt.bfloat16
    I16 = mybir.dt.int16
    Exp = mybir.ActivationFunctionType.Exp
    Ident = mybir.ActivationFunctionType.Identity
    MUL = mybir.AluOpType.mult
    ADD = mybir.AluOpType.add

    # Schraudolph: e^(l/8) = 2^(l*0.125*log2 e); bf16 bits =
    # 128*(z+127) - 5.5 (centers the pwl error) + 0.5 (truncation)
    FE_A = 0.125 * 1.4426950408889634 * 128.0
    FE_B = 127.0 * 128.0 - 5.5 + 0.5

    nc = bass.Bass("TRN2", target_bir_lowering=False, debug=False,
                   num_devices=NCORES)
    xbd = nc.dram_tensor("xb", [D, S], BF16, kind="ExternalInput")
    wqbd = nc.dram_tensor("wqb", [D, G], BF16, kind="ExternalInput")
    wkbd = nc.dram_tensor("wkb", [D, G], BF16, kind="ExternalInput")
    wvbd = nc.dram_tensor("wvb", [D, G], BF16, kind="ExternalInput")
    wod = nc.dram_tensor("wo", [G, D], BF16, kind="ExternalInput")
    bqd = nc.dram_tensor("bq", [G], F32, kind="ExternalInput")
    outd = nc.dram_tensor("out", [S, D], F32, kind="ExternalOutput")

    with tile.TileContext(nc) as tc:
        with tc.tile_pool(name="persist", bufs=1) as pp, \
             tc.tile_pool(name="dram", bufs=1, space="DRAM") as dp:
            qts = pp.tile([128, TT, S], BF16, tag="qts")
            kts = pp.tile([128, TT, S], BF16, tag="kts")
            vsb = pp.tile([128, ST, 8, HD + 1], BF16, tag="vsb")
            valsT = pp.tile([128, TT, S], BF16, tag="valsT")
            wos = pp.tile([128, TT, D], BF16, tag="wos")
            bqt = pp.tile([128, TT], F32, tag="bqt")

            nc.vector.memset(vsb[:, :, :, HD:HD + 1], 1.0)

            # ---- Phase 1: projections (bf16) ----
            with tc.tile_pool(name="proj", bufs=1) as jp, \
                 tc.tile_pool(name="ps_proj", bufs=8, space="PSUM") as psp:
                xbs = jp.tile([128, KT_, S], BF16, tag="xbs")
                wqbs = jp.tile([128, KT_, G], BF16, tag="wqbs")
                wkbs = jp.tile([128, KT_, G], BF16, tag="wkbs")
                wvbs = jp.tile([128, KT_, G], BF16, tag="wvbs")
                # stagger loads so K-proj never outruns the x stream
                for k in range(KT_):
                    nc.sync.dma_start(out=wkbs[:, k, :], in_=wkbd[k * 128:(k + 1) * 128, :])
                    nc.sync.dma_start(out=xbs[:, k, :], in_=xbd[k * 128:(k + 1) * 128, :])
                for k in range(KT_):
                    nc.sync.dma_start(out=wqbs[:, k, :], in_=wqbd[k * 128:(k + 1) * 128, :])
                for k in range(KT_):
                    nc.sync.dma_start(out=wvbs[:, k, :], in_=wvbd[k * 128:(k + 1) * 128, :])
                for t in range(TT):
                    nc.sync.dma_start(out=wos[:, t, :], in_=wod[t * 128:(t + 1) * 128, :])
                # bq host-pretransposed to [128, 4] p-major; small gather
                # stays off the sync queue and behind the big loads
                nc.gpsimd.dma_start(
                    out=bqt, in_=bqd.rearrange("(p t) -> p t", t=TT))

                # K^T then Q^T: weight-stationary, 2-t-tile groups, k-outer
                for which, ws in (("k", wkbs), ("q", wqbs)):
                    for tg in range(2):
                        pss = [[psp.tile([128, 512], F32, tag="pj", name="pj")
                                for _ in range(4)] for _ in range(2)]
                        for k in range(KT_):
                            for ti in range(2):
                                t = 2 * tg + ti
                                for sc in range(4):
                                    nc.tensor.matmul(
                                        pss[ti][sc],
                                        ws[:, k, t * 128:(t + 1) * 128],
                                        xbs[:, k, sc * 512:(sc + 1) * 512],
                                        start=(k == 0), stop=(k == KT_ - 1))
                        for ti in range(2):
                            t = 2 * tg + ti
                            for sc in range(4):
                                ps_, sl = pss[ti][sc], slice(sc * 512, (sc + 1) * 512)
                                if which == "q":
                                    nc.scalar.activation(
                                        qts[:, t, sl], ps_, Ident,
                                        bias=bqt[:, t:t + 1], scale=1.0)
                                else:
                                    nc.vector.tensor_copy(out=kts[:, t, sl], in_=ps_)

                # V: x-stationary; drains on ScalarE (idle in proj)
                for s_ in range(ST):
                    ps = psp.tile([128, 512], F32, tag="pj", name="pj")
                    for k in range(KT_):
                        nc.tensor.matmul(
                            ps, xbs[:, k, s_ * 128:(s_ + 1) * 128],
                            wvbs[:, k, :],
                            start=(k == 0), stop=(k == KT_ - 1))
                    nc.scalar.copy(
                        out=vsb[:, s_, :, 0:HD],
                        in_=ps.rearrange("p (h d) -> p h d", h=8))

            # ---- Phase 2: attention + output projection ----
            with tc.tile_pool(name="ptp", bufs=4) as ptp, \
                 tc.tile_pool(name="bcp", bufs=2) as bcp, \
                 tc.tile_pool(name="smp", bufs=2) as smp, \
                 tc.tile_pool(name="vshift", bufs=2) as vsp, \
                 tc.tile_pool(name="outp", bufs=2) as op_, \
                 tc.tile_pool(name="ps_lg", bufs=2, space="PSUM") as lgp, \
                 tc.tile_pool(name="ps_pv", bufs=2, space="PSUM") as pvp:

                def chain_srow_half(state, hf):
                    """copy half the sums row (PSUM -> SBUF, ScalarE)"""
                    pv_t = state["pv"]
                    if "srow_sb" not in state:
                        state["srow_sb"] = smp.tile([1, SBW], F32, tag="srow_sb", name="srow_sb")
                        state["srow"] = dp.tile([SBW], F32, tag="srow", bufs=3, name="srow")
                    sl = slice(hf * 512, (hf + 1) * 512)
                    nc.vector.tensor_copy(out=state["srow_sb"][:, sl], in_=pv_t[64:65, sl])
                    nc.sync.dma_start(
                        out=state["srow"].rearrange("(a b) -> a b", a=1)[:, sl],
                        in_=state["srow_sb"][:, sl])

                def chain_recip(state):
                    ssb = smp.tile([128, SBW // 128], F32, tag="ssb")
                    nc.sync.dma_start(
                        out=ssb, in_=state["srow"].rearrange("(p f) -> p f", p=128))
                    rsb = smp.tile([128, SBW // 128], F32, tag="rsb")
                    nc.vector.reciprocal(out=rsb, in_=ssb)
                    rrow = dp.tile([SBW], F32, tag="rrow", bufs=3)
                    nc.sync.dma_start(
                        out=rrow.rearrange("(p f) -> p f", p=128), in_=rsb)
                    bc = bcp.tile([64, SBW], F32, tag="bc")
                    nc.sync.dma_start(
                        out=bc,
                        in_=rrow.rearrange("(a b) -> a b", a=1)
                                .partition_broadcast(64))
                    state["bc"] = bc

                def chain_norm_half(state, hf):
                    """normalize-multiply half on DVE (+ stage odd heads)"""
                    h, blk, pv_t, bc = state["h"], state["blk"], state["pv"], state["bc"]
                    t, p0 = h // 2, (h % 2) * 64
                    s0b = blk * SBW
                    sl = slice(hf * 512, (hf + 1) * 512)
                    osl = slice(s0b + hf * 512, s0b + (hf + 1) * 512)
                    if p0 == 0:
                        nc.vector.tensor_mul(
                            valsT[0:64, t, osl], pv_t[0:64, sl], bc[:, sl])
                    else:
                        if "vs" not in state:
                            state["vs"] = vsp.tile([64, SBW], BF16, tag="vs", name="vs")
                        nc.vector.tensor_mul(
                            state["vs"][:, sl], pv_t[0:64, sl], bc[:, sl])
                        if hf == 1:
                            nc.gpsimd.dma_start(
                                out=valsT[64:128, t, s0b:s0b + SBW],
                                in_=state["vs"])

                def outproj_mm(state, half):
                    """output projection si-tile: 4 accumulating matmuls"""
                    if state.get("ops") is None:
                        state["ops"] = lgp.tile([128, SBW], F32, tag="lg", name="ops")
                    s0 = state["blk"] * SBW + state["st"] * 128
                    for gt in range(TT):
                        nc.tensor.matmul(
                            state["ops"][:, half * 512:(half + 1) * 512],
                            valsT[:, gt, s0:s0 + 128],
                            wos[:, gt, half * 512:(half + 1) * 512],
                            start=(gt == 0), stop=(gt == TT - 1))

                def outproj_drain_half(state, hf):
                    if state.get("ob") is None:
                        state["ob"] = op_.tile([128, D], F32, tag="ob", name="ob")
                    sl = slice(hf * 512, (hf + 1) * 512)
                    nc.vector.tensor_copy(out=state["ob"][:, sl], in_=state["ops"][:, sl])
                    if hf == 1:
                        s0 = state["blk"] * SBW + state["st"] * 128
                        nc.sync.dma_start(out=outd[s0:s0 + 128, :], in_=state["ob"])

                pending_pv = []        # PV emitters, two units behind logits
                chain = None           # state dict for previous head's chain
                outproj_q = []         # deferred outproj units (st, blk)
                op_state = None
                for hs in range(2 * 8):
                    blk, h = hs // 8, hs % 8
                    s0b = blk * SBW
                    t, p0 = h // 2, (h % 2) * 64
                    qrow = qts[p0:p0 + 64, t, s0b:s0b + SBW]
                    pv = pvp.tile([65, SBW], F32, tag="pv")
                    for sj in range(ST):
                        lg = lgp.tile([128, SBW], F32, tag="lg")
                        lkt = kts[p0:p0 + 64, t, sj * 128:(sj + 1) * 128]
                        for hf in range(2):
                            nc.tensor.matmul(
                                lg[:, hf * 512:(hf + 1) * 512],
                                lkt, qrow[:, hf * 512:(hf + 1) * 512],
                                start=True, stop=True)
                        pt = ptp.tile([128, SBW], BF16, tag="pt")
                        # full-tile exp (per-instr overhead ~400ns makes
                        # half-splitting a net loss): ScalarE table-exp on
                        # 13 of 16 tiles, DVE Schraudolph on {3, 8, 13}.
                        # ScalarE stays pure-Exp in attention (no
                        # activation-table reloads).
                        if sj in (3, 8, 13):
                            nc.vector.tensor_scalar(
                                pt.bitcast(I16), lg, FE_A, FE_B, MUL, ADD)
                        else:
                            nc.scalar.activation(pt, lg, Exp, scale=0.125)
                        def _pv(pv=pv, pt=pt, sj=sj, h=h):
                            lv = vsb[:, sj, h, 0:HD + 1]
                            for hf in range(2):
                                nc.tensor.matmul(
                                    pv[:, hf * 512:(hf + 1) * 512],
                                    lv, pt[:, hf * 512:(hf + 1) * 512],
                                    start=(sj == 0), stop=(sj == ST - 1))
                        pending_pv.append(_pv)
                        if len(pending_pv) > 2:
                            pending_pv.pop(0)()
                        # previous head's normalize chain, spread thin
                        if chain is not None:
                            if sj == 5:
                                chain_srow_half(chain, 0)
                            elif sj == 7:
                                chain_srow_half(chain, 1)
                            elif sj == 10:
                                chain_recip(chain)
                            elif sj == 14:
                                chain_norm_half(chain, 0)
                            elif sj == 15:
                                chain_norm_half(chain, 1)
                                if chain["h"] == 7 and chain["blk"] == 0:
                                    outproj_q = [(st, 0) for st in range(SBW // 128)]
                                chain = None
                        # drip the deferred output projection, split thin
                        if sj == 1 and outproj_q and op_state is None:
                            st_, blk_ = outproj_q.pop(0)
                            op_state = {"st": st_, "blk": blk_}
                            outproj_mm(op_state, 0)
                        elif sj == 4 and op_state is not None:
                            outproj_mm(op_state, 1)
                        elif sj == 6 and op_state is not None:
                            outproj_drain_half(op_state, 0)
                        elif sj == 9 and op_state is not None:
                            outproj_drain_half(op_state, 1)
                            op_state = None
                    chain = {"h": h, "blk": blk, "pv": pv}
                for f in pending_pv:   # flush the last two PV units
                    f()
                pending_pv = []
                # tail: last head's chain, then the last block's outproj
                chain_srow_half(chain, 0)
                chain_srow_half(chain, 1)
                chain_recip(chain)
                chain_norm_half(chain, 0)
                chain_norm_half(chain, 1)
                tail_units = ([{"st": st_, "blk": blk_} for st_, blk_ in outproj_q]
                              + [{"st": st, "blk": 1} for st in range(SBW // 128)])
                for st_state in tail_units:
                    outproj_mm(st_state, 0)
                    outproj_mm(st_state, 1)
                    outproj_drain_half(st_state, 0)
                    outproj_drain_half(st_state, 1)

    _split_sync_waits(nc, mybir)
    return nc


def _get_nc():
    if "nc" not in _cache:
        _cache["nc"] = _build()
    return _cache["nc"]


def _run(in_maps, **kw):
    from concourse.bass_utils import run_bass_kernel_spmd
    return run_bass_kernel_spmd(_get_nc(), in_maps, core_ids=list(range(NCORES)), **kw)


def _make_in_maps(x, Wq, bq, Wk, bk, Wv, bv, Wo, bo):
    import ml_dtypes
    BF = ml_dtypes.bfloat16
    x = np.asarray(x, np.float32)
    in_maps = []
    for c in range(NCORES):
        b, g = c // 2, c % 2
        gs = slice(g * G, (g + 1) * G)
        in_maps.append({
            "xb": np.ascontiguousarray(x[b].T).astype(BF),
            "wqb": np.ascontiguousarray(np.asarray(Wq, np.float32)[:, gs]).astype(BF),
            "wkb": np.ascontiguousarray(np.asarray(Wk, np.float32)[:, gs]).astype(BF),
            "wvb": np.ascontiguousarray(np.asarray(Wv, np.float32)[:, gs]).astype(BF),
            "wo": np.ascontiguousarray(np.asarray(Wo, np.float32)[gs, :]).astype(BF),
            # pre-transposed to [128 partitions, 4 t-tiles] p-major
            "bq": np.ascontiguousarray(
                np.asarray(bq, np.float32)[gs].reshape(TT, 128).T),
        })
    return in_maps


def kernel(x, Wq, bq, Wk, bk, Wv, bv, Wo, bo, **_kw):
    res = _run(_make_in_maps(x, Wq, bq, Wk, bk, Wv, bv, Wo, bo))
    # host-side bias: bv @ Wo + bo (exact: softmax rows sum to 1)
    hb = (np.asarray(bv, np.float64) @ np.asarray(Wo, np.float64)
          + np.asarray(bo, np.float64)).astype(np.float32)
    out = np.empty((B, S, D), dtype=np.float32)
    for b in range(B):
        out[b] = res.results[2 * b]["out"] + res.results[2 * b + 1]["out"] + hb
    return out
